# revision 1
# baseline (speedup 1.0000x reference)
"""Trainium2 Bass kernel for nn_CantorModalityFusion.

Sharding: 8 cores = (batch b in 0..3) x (position parity in 0..1).
Each core handles batch b, positions s = par, par+2, ... (1024 positions).
The computation is per-(b, s) independent -> no collectives.

Device layout is feature-major: activations live as [feature, position]
tiles so every matmul has its contraction dim on SBUF partitions and
host-pre-transposed weights stream in naturally. The host transposes
inputs / un-transposes outputs (layout only, no math).

Pipeline per 512-position block:
  A:  p.T[m] = Wm.T.T @ x.T[m] (+ b_m + mod_emb[m])          [PE+ACT]
  B1: q.T/k.T per feature chunk; s_w += sel.T @ (q*k)        [PE+DVE]
  SM: softmax over the 3 routed windows                      [DVE+ACT]
  B2: v.T per chunk; A16_r = sum attn; Abc = bcast(A16);
      fused.T[c] = sum_r Abc_r * v.T[r]                      [PE+DVE]
  D:  y.T = Wo.T.T @ fused.T (+ bo)                          [PE+ACT]
"""

import sys

import numpy as np

sys.path.insert(0, "/opt/trn_rl_repo")

import concourse.bacc as bacc
import concourse.mybir as mybir
from concourse import tile
from concourse.bass_utils import run_bass_kernel_spmd

F32 = mybir.dt.float32
F32R = mybir.dt.float32r
AF = mybir.ActivationFunctionType
ALU = mybir.AluOpType

B, S, D, H, HD = 4, 2048, 1024, 16, 64
M, WIN = 4, 3
MOD = [("text", 768, 2048), ("image", 1024, 1024), ("audio", 512, 1500), ("video", 2048, 512)]
ROUTES = [[0, 1, 2], [0, 1, 2], [2, 3, 0], [3, 2, 0]]
PAIRS = [(m, w, ROUTES[m][w]) for m in range(M) for w in range(WIN)]
SRC = {r: [(m, w) for (m, w, rr) in PAIRS if rr == r] for r in range(M)}
# flat index for selA constant: (m, w) -> 0..11
PAIR_IDX = {(m, w): m * WIN + w for m in range(M) for w in range(WIN)}

NPOS = S // 2
BLK = 512
NBLK = NPOS // BLK
NCH = D // 128                       # 8 feature chunks
NLOC = [sl // 2 for (_, _, sl) in MOD]   # 1024, 512, 750, 256

_BUILD_CACHE = {}


def n_active(m, blk):
    return max(0, min(BLK, NLOC[m] - blk * BLK))


def build(scale, repeat=1):
    key = (float(scale), repeat)
    if key in _BUILD_CACHE:
        return _BUILD_CACHE[key]
    nc = bacc.Bacc("TRN2", target_bir_lowering=False, debug=False)

    xT = [nc.dram_tensor(f"xT{m}", [MOD[m][1], NLOC[m]], F32R, kind="ExternalInput")
          for m in range(M)]
    WmT = [nc.dram_tensor(f"WmT{m}", [MOD[m][1], D], F32R, kind="ExternalInput")
           for m in range(M)]
    WqT = nc.dram_tensor("WqT", [NCH, 128, NCH, 128], F32R, kind="ExternalInput")
    WkT = nc.dram_tensor("WkT", [NCH, 128, NCH, 128], F32R, kind="ExternalInput")
    WvT = nc.dram_tensor("WvT", [NCH, 128, NCH, 128], F32R, kind="ExternalInput")
    WoT = nc.dram_tensor("WoT", [NCH, 128, NCH, 128], F32R, kind="ExternalInput")
    bm_d = nc.dram_tensor("bm", [128, M, NCH], F32, kind="ExternalInput")
    emb_d = nc.dram_tensor("emb", [128, M, NCH], F32, kind="ExternalInput")
    bq_d = nc.dram_tensor("bq", [128, NCH], F32, kind="ExternalInput")
    bk_d = nc.dram_tensor("bk", [128, NCH], F32, kind="ExternalInput")
    bv_d = nc.dram_tensor("bv", [128, NCH], F32, kind="ExternalInput")
    bo_d = nc.dram_tensor("bo", [128, NCH], F32, kind="ExternalInput")
    selw_d = nc.dram_tensor("selw", [128, 127], F32R, kind="ExternalInput")
    selA_d = nc.dram_tensor("selA", [64, M * WIN, 16], F32R, kind="ExternalInput")
    selB_d = nc.dram_tensor("selB", [16, NCH, 128], F32R, kind="ExternalInput")
    yT = nc.dram_tensor("yT", [D, NPOS], F32, kind="ExternalOutput")

    with tile.TileContext(nc) as tc:
        with (
            tc.tile_pool(name="const", bufs=1) as cpool,
            tc.tile_pool(name="wa", bufs=3) as wapool,
            tc.tile_pool(name="wb", bufs=3) as wbpool,
            tc.tile_pool(name="xt", bufs=3) as xtpool,
            tc.tile_pool(name="pt", bufs=1) as ptpool,
            tc.tile_pool(name="qk", bufs=1) as qkpool,
            tc.tile_pool(name="pr", bufs=2) as prpool,
            tc.tile_pool(name="sm", bufs=1) as smpool,
            tc.tile_pool(name="fz", bufs=1) as fzpool,
            tc.tile_pool(name="yo", bufs=2) as yopool,
            tc.tile_pool(name="ps", bufs=1, space="PSUM") as pspool,
        ):
            def psum(i, shape=(128, BLK)):
                return pspool.tile(list(shape), F32, tag=f"a{i}", name=f"ps_a{i}")

            # ---- constants ----
            selw = cpool.tile([128, 127], F32R, tag="selw")
            nc.sync.dma_start(selw[:], selw_d[:])
            selA = cpool.tile([64, M * WIN, 16], F32R, tag="selA")
            nc.sync.dma_start(selA[:], selA_d[:])
            selB = cpool.tile([16, NCH, 128], F32R, tag="selB")
            nc.sync.dma_start(selB[:], selB_d[:])
            bmt = cpool.tile([128, M, NCH], F32, tag="bmt")
            nc.sync.dma_start(bmt[:], bm_d[:])
            embt = cpool.tile([128, M, NCH], F32, tag="embt")
            nc.sync.dma_start(embt[:], emb_d[:])
            biasA = cpool.tile([128, M, NCH], F32, tag="biasA")
            nc.vector.tensor_add(biasA[:], bmt[:], embt[:])
            bqkvo = {}
            for nm, dd in (("bq", bq_d), ("bk", bk_d), ("bv", bv_d), ("bo", bo_d)):
                t = cpool.tile([128, NCH], F32, tag=nm)
                nc.sync.dma_start(t[:], dd[:])
                bqkvo[nm] = t

            import contextlib
            rep_cm = (tc.For_i(0, repeat, 1,
                               hint_engines=(mybir.EngineType.PE,
                                             mybir.EngineType.Activation,
                                             mybir.EngineType.DVE,
                                             mybir.EngineType.SP,
                                             mybir.EngineType.Pool))
                      if repeat > 1 else contextlib.nullcontext())
            pending_D = []
            with rep_cm:
                for blk in range(NBLK):
                    nact = [n_active(m, blk) for m in range(M)]
                    act_m = [m for m in range(M) if nact[m] > 0]
                    act_pairs = [(m, w, r) for (m, w, r) in PAIRS
                                 if nact[m] > 0 and nact[r] > 0]
                    p0 = blk * BLK

                    # ---------- stage A ----------
                    pT = {}
                    for m in act_m:
                        na = nact[m]
                        nk = MOD[m][1] // 128
                        pt = ptpool.tile([128, NCH, BLK], F32R, tag=f"pt{m}")
                        pT[m] = pt
                        if na < BLK:
                            nc.gpsimd.memset(pt[:, :, na:BLK].bitcast(F32), 0.0)
                        accs = [psum(i) for i in range(NCH)]
                        for dk in range(nk):
                            deng = nc.sync if dk % 2 == 0 else nc.scalar
                            wrow = wapool.tile([128, D], F32R, tag="wa")
                            deng.dma_start(
                                wrow[:], WmT[m][dk * 128:(dk + 1) * 128, :])
                            xt_t = xtpool.tile([128, BLK], F32R, tag="xt")
                            deng.dma_start(
                                xt_t[:, :na],
                                xT[m][dk * 128:(dk + 1) * 128, p0:p0 + na])
                            for dc in range(NCH):
                                nc.tensor.matmul(
                                    accs[dc][:, :na],
                                    wrow[:, dc * 128:(dc + 1) * 128],
                                    xt_t[:, :na],
                                    start=(dk == 0), stop=(dk == nk - 1),
                                    skip_group_check=True)
                        for dc in range(NCH):
                            nc.scalar.activation(
                                pt[:, dc, :na], accs[dc][:, :na], AF.Identity,
                                bias=biasA[:, m, dc:dc + 1])

                    # ---------- pass 1: q, k, scores ----------
                    n_sc = {w: sum(1 for (m, w2, r) in act_pairs if w2 == w) * NCH
                            for w in range(WIN)}
                    c_sc = {w: 0 for w in range(WIN)}
                    sc_ps = [psum(5 + w, (64, BLK)) if n_sc[w] > 0 else None
                             for w in range(WIN)]
                    def emit_qk(c):
                        qk_t = {}
                        for ti, (tname, wd, bias) in enumerate(
                                (("q", WqT, bqkvo["bq"]),
                                 ("k", WkT, bqkvo["bk"]))):
                            wsl = wbpool.tile([128, NCH, 128], F32R, tag="wb",
                                              name="wsl")
                            nc.sync.dma_start(wsl[:], wd[c])
                            for mi, m in enumerate(act_m):
                                acc = psum((2 * c + ti + mi) % 3)
                                for dk in range(NCH):
                                    nc.tensor.matmul(
                                        acc[:], wsl[:, dk, :], pT[m][:, dk, :],
                                        start=(dk == 0), stop=(dk == NCH - 1),
                                        skip_group_check=True)
                                t = qkpool.tile([128, BLK], F32,
                                                tag=f"{tname}{m}_{c % 2}",
                                                name=f"{tname}{m}")
                                nc.scalar.activation(
                                    t[:], acc[:], AF.Identity,
                                    bias=bias[:, c:c + 1])
                                qk_t[(tname, m)] = t
                        return qk_t

                    def emit_scores(c, qk_t):
                        for pi, (m, w, r) in enumerate(act_pairs):
                            prod = prpool.tile([128, BLK], F32R, bufs=1,
                                               tag=f"prod{pi}", name="prod")
                            nc.vector.tensor_mul(
                                prod[:], qk_t[("q", m)][:], qk_t[("k", r)][:])
                            off = 62 - (16 * m + 2 * c)
                            i = c_sc[w]
                            c_sc[w] += 1
                            nc.tensor.matmul(
                                sc_ps[w][:], selw[:, off:off + 64], prod[:],
                                start=(i == 0), stop=(i == n_sc[w] - 1),
                                skip_group_check=True)

                    prev = emit_qk(0)
                    for c in range(1, NCH):
                        cur = emit_qk(c)
                        emit_scores(c - 1, prev)
                        prev = cur
                    emit_scores(NCH - 1, prev)
                    if pending_D:
                        emit_stage_D(*pending_D.pop(0))

                    # ---------- softmax ----------
                    s_sb = []
                    for w in range(WIN):
                        t = smpool.tile([64, BLK], F32, tag=f"s{w}")
                        if sc_ps[w] is None:
                            nc.gpsimd.memset(t[:], 0.0)
                        else:
                            nc.vector.tensor_copy(t[:], sc_ps[w][:])
                        s_sb.append(t)
                    mx = smpool.tile([64, BLK], F32, tag="mx")
                    nc.vector.tensor_tensor(mx[:], s_sb[0][:], s_sb[1][:], op=ALU.max)
                    nc.vector.tensor_tensor(mx[:], mx[:], s_sb[2][:], op=ALU.max)
                    attn = []
                    for w in range(WIN):
                        nc.vector.tensor_tensor(s_sb[w][:], s_sb[w][:], mx[:],
                                                op=ALU.subtract)
                        a = smpool.tile([64, BLK], F32R, tag=f"at{w}")
                        nc.scalar.activation(a[:], s_sb[w][:], AF.Exp, scale=scale)
                        attn.append(a)
                    den = smpool.tile([64, BLK], F32, tag="mx")
                    nc.vector.tensor_add(den[:], attn[0][:], attn[1][:])
                    nc.vector.tensor_add(den[:], den[:], attn[2][:])
                    rec = smpool.tile([64, BLK], F32R, tag="rec")
                    with nc.allow_low_precision(reason="fp32r attn weights"):
                        nc.vector.reciprocal(rec[:], den[:])
                    for w in range(WIN):
                        nc.vector.tensor_mul(attn[w][:], attn[w][:], rec[:])

                    # ---------- pass 2: v, A16, Abc, fused ----------
                    act_r = [r for r in range(M) if nact[r] > 0]
                    fz = fzpool.tile([128, NCH, BLK], F32R, tag="fz")

                    def emit_v(c):
                        wsl = wbpool.tile([128, NCH, 128], F32R, tag="wb",
                                          name="wslv")
                        nc.sync.dma_start(wsl[:], WvT[c])
                        v_t = {}
                        for mi, m in enumerate(act_m):
                            acc = psum([0, 5, 6][mi % 3])
                            for dk in range(NCH):
                                nc.tensor.matmul(
                                    acc[:], wsl[:, dk, :], pT[m][:, dk, :],
                                    start=(dk == 0), stop=(dk == NCH - 1),
                                    skip_group_check=True)
                            t = qkpool.tile([128, BLK], F32,
                                            tag=f"q{m}_{c % 2}", name="vt")
                            nc.scalar.activation(
                                t[:], acc[:], AF.Identity,
                                bias=bqkvo["bv"][:, c:c + 1])
                            v_t[m] = t
                        return v_t

                    def emit_fused(c, v_t, a16sb):
                        ab_ps = {}
                        for r in act_r:
                            ab = psum(1 + r)
                            nc.tensor.matmul(
                                ab[:], selB[:, c, :], a16sb[:, r, :],
                                start=True, stop=True,
                                skip_group_check=True)
                            ab_ps[r] = ab
                        r0 = act_r[0]
                        accv = prpool.tile([128, BLK], F32, tag="f0",
                                           name="accv")
                        nc.vector.tensor_mul(accv[:], ab_ps[r0][:], v_t[r0][:])
                        if len(act_r) == 1:
                            nc.vector.tensor_copy(fz[:, c, :], accv[:])
                        for j, r in enumerate(act_r[1:]):
                            tmp = prpool.tile([128, BLK], F32, tag="f1",
                                              name="tmp")
                            nc.vector.tensor_mul(tmp[:], ab_ps[r][:], v_t[r][:])
                            last = (j == len(act_r) - 2)
                            nc.vector.tensor_add(
                                fz[:, c, :] if last else accv[:],
                                accv[:], tmp[:])

                    vbuf = {0: emit_v(0)}
                    if NCH > 1:
                        vbuf[1] = emit_v(1)

                    # A16 = per-source summed attn (waits on softmax; emitted
                    # after two v chunks so the PE stream has work meanwhile)
                    a16sb = smpool.tile([16, M, BLK], F32R, tag="a16sb")
                    for r in act_r:
                        a16 = psum(1 + r, (16, BLK))
                        srcs = SRC[r]
                        for i, (m, w) in enumerate(srcs):
                            nc.tensor.matmul(
                                a16[:], selA[:, PAIR_IDX[(m, w)], :],
                                attn[w][:],
                                start=(i == 0), stop=(i == len(srcs) - 1),
                                skip_group_check=True)
                        nc.scalar.activation(a16sb[:, r, :], a16[:], AF.Identity)

                    for c in range(NCH):
                        emit_fused(c, vbuf.pop(c), a16sb)
                        if c + 2 < NCH:
                            vbuf[c + 2] = emit_v(c + 2)

                    # ---------- stage D (deferred) ----------
                    def emit_stage_D(fz, p0):
                        for dc in range(NCH):
                            wsl = wbpool.tile([128, NCH, 128], F32R,
                                              tag="wb", name="wsld")
                            nc.sync.dma_start(wsl[:], WoT[dc])
                            acc = psum(dc % 3)
                            for dk in range(NCH):
                                nc.tensor.matmul(
                                    acc[:], wsl[:, dk, :], fz[:, dk, :],
                                    start=(dk == 0), stop=(dk == NCH - 1),
                                    skip_group_check=True)
                            yo = yopool.tile([128, BLK], F32, tag="yo")
                            nc.scalar.activation(yo[:], acc[:], AF.Identity,
                                                 bias=bqkvo["bo"][:, dc:dc + 1])
                            nc.sync.dma_start(
                                yT[dc * 128:(dc + 1) * 128, p0:p0 + BLK], yo[:])

                    pending_D.append((fz, p0))

                    if blk == NBLK - 1:
                        while pending_D:
                            emit_stage_D(*pending_D.pop(0))


    nc.compile()
    _BUILD_CACHE[key] = nc
    return nc


def make_selw():
    sw = np.zeros((128, 127), np.float32)
    for p in range(128):
        sw[p, 62 + p // 64] = 1.0
    return sw


def make_selA():
    sa = np.zeros((64, M * WIN, 16), np.float32)
    for m in range(M):
        for w in range(WIN):
            for h in range(16):
                sa[16 * m + h, m * WIN + w, h] = 1.0
    return sa


def make_selB():
    sb = np.zeros((16, NCH, 128), np.float32)
    for c in range(NCH):
        for j in range(128):
            sb[2 * c + j // 64, c, j] = 0.25
    return sb


def _vec_tile(v):
    return np.ascontiguousarray(np.asarray(v, np.float32).reshape(NCH, 128).T)


def prepare_in_maps(inputs):
    names = [mm[0] for mm in MOD]
    shared = {}
    for i, nm in enumerate(names):
        shared[f"WmT{i}"] = np.ascontiguousarray(
            np.asarray(inputs[f"W_{nm}"], np.float32).T)
    for k, srcn in (("WqT", "Wq"), ("WkT", "Wk"), ("WvT", "Wv"), ("WoT", "Wo")):
        wt = np.asarray(inputs[srcn], np.float32).T      # [din, dout]
        wt = wt.reshape(NCH, 128, NCH, 128)              # [dk, p, c, j]
        shared[k] = np.ascontiguousarray(wt.transpose(2, 1, 0, 3))  # [c, p, dk, j]
    shared["bm"] = np.ascontiguousarray(
        np.stack([_vec_tile(inputs[f"b_{nm}"]) for nm in names], axis=1))
    shared["emb"] = np.ascontiguousarray(
        np.stack([_vec_tile(np.asarray(inputs["mod_emb"])[i]) for i in range(M)],
                 axis=1))
    for k in ("bq", "bk", "bv", "bo"):
        shared[k] = _vec_tile(inputs[k])
    shared["selw"] = make_selw()
    shared["selA"] = make_selA()
    shared["selB"] = make_selB()

    in_maps = []
    for core in range(8):
        b, par = core // 2, core % 2
        im = dict(shared)
        for i, nm in enumerate(names):
            x = np.asarray(inputs[nm], np.float32)[b, par::2][:NLOC[i]]
            im[f"xT{i}"] = np.ascontiguousarray(x.T)
        in_maps.append(im)
    return in_maps


def kernel(**inputs):
    inputs = {k: np.asarray(v) for k, v in inputs.items()}
    scale = float(1.0 / (np.sqrt(HD) * abs(float(inputs["temperature"]))))
    nc = build(scale, repeat=1)
    in_maps = prepare_in_maps(inputs)
    res = run_bass_kernel_spmd(nc, in_maps, list(range(8)))
    out = np.zeros((B, S, D), np.float32)
    for core in range(8):
        b, par = core // 2, core % 2
        out[b, par::2, :] = res.results[core]["yT"].T
    return out



# revision 7
# speedup vs baseline: 1.5494x; 1.5494x over previous
"""Trainium2 Bass kernel for nn_CantorModalityFusion.

Sharding: 8 cores = (batch b in 0..3) x (position parity in 0..1).
Each core handles batch b, positions s = par, par+2, ... (1024 positions).
The computation is per-(b, s) independent -> no collectives.

Device layout is feature-major: activations live as [feature, position]
tiles so every matmul has its contraction dim on SBUF partitions and
host-pre-transposed weights stream in naturally. The host transposes
inputs / un-transposes outputs (layout only, no math).

Pipeline per 512-position block:
  A:  p.T[m] = Wm.T.T @ x.T[m] (+ b_m + mod_emb[m])          [PE+ACT]
  B1: q.T/k.T per feature chunk; s_w += sel.T @ (q*k)        [PE+DVE]
  SM: softmax over the 3 routed windows                      [DVE+ACT]
  B2: v.T per chunk; A16_r = sum attn; Abc = bcast(A16);
      fused.T[c] = sum_r Abc_r * v.T[r]                      [PE+DVE]
  D:  y.T = Wo.T.T @ fused.T (+ bo)                          [PE+ACT]
"""

import sys

import numpy as np

sys.path.insert(0, "/opt/trn_rl_repo")

import concourse.bacc as bacc
import concourse.mybir as mybir
from concourse import tile
from concourse.bass_utils import run_bass_kernel_spmd

F32 = mybir.dt.float32
F32R = mybir.dt.float32r
BF16 = mybir.dt.float16
AF = mybir.ActivationFunctionType
ALU = mybir.AluOpType

B, S, D, H, HD = 4, 2048, 1024, 16, 64
M, WIN = 4, 3
MOD = [("text", 768, 2048), ("image", 1024, 1024), ("audio", 512, 1500), ("video", 2048, 512)]
ROUTES = [[0, 1, 2], [0, 1, 2], [2, 3, 0], [3, 2, 0]]
PAIRS = [(m, w, ROUTES[m][w]) for m in range(M) for w in range(WIN)]
SRC = {r: [(m, w) for (m, w, rr) in PAIRS if rr == r] for r in range(M)}
# flat index for selA constant: (m, w) -> 0..11
PAIR_IDX = {(m, w): m * WIN + w for m in range(M) for w in range(WIN)}

NPOS = S // 2
BLK = 512
NBLK = NPOS // BLK
NCH = D // 128                       # 8 feature chunks
NLOC = [sl // 2 for (_, _, sl) in MOD]   # 1024, 512, 750, 256

_BUILD_CACHE = {}


def n_active(m, blk):
    return max(0, min(BLK, NLOC[m] - blk * BLK))


def build(scale, repeat=1):
    key = (float(scale), repeat)
    if key in _BUILD_CACHE:
        return _BUILD_CACHE[key]
    nc = bacc.Bacc("TRN2", target_bir_lowering=False, debug=False)

    xT = [nc.dram_tensor(f"xT{m}", [MOD[m][1], NLOC[m]], BF16, kind="ExternalInput")
          for m in range(M)]
    WmT = [nc.dram_tensor(f"WmT{m}", [MOD[m][1], D], BF16, kind="ExternalInput")
           for m in range(M)]
    WqT = nc.dram_tensor("WqT", [NCH, 128, NCH, 128], BF16, kind="ExternalInput")
    WkT = nc.dram_tensor("WkT", [NCH, 128, NCH, 128], BF16, kind="ExternalInput")
    WvT = nc.dram_tensor("WvT", [NCH, 128, NCH, 128], BF16, kind="ExternalInput")
    WoT = nc.dram_tensor("WoT", [NCH, 128, NCH, 128], BF16, kind="ExternalInput")
    bm_d = nc.dram_tensor("bm", [128, M, NCH], F32, kind="ExternalInput")
    emb_d = nc.dram_tensor("emb", [128, M, NCH], F32, kind="ExternalInput")
    bq_d = nc.dram_tensor("bq", [128, NCH], F32, kind="ExternalInput")
    bk_d = nc.dram_tensor("bk", [128, NCH], F32, kind="ExternalInput")
    bv_d = nc.dram_tensor("bv", [128, NCH], F32, kind="ExternalInput")
    bo_d = nc.dram_tensor("bo", [128, NCH], F32, kind="ExternalInput")
    selw_d = nc.dram_tensor("selw", [128, 127], BF16, kind="ExternalInput")
    selA_d = nc.dram_tensor("selA", [64, M * WIN, 16], BF16, kind="ExternalInput")
    selB_d = nc.dram_tensor("selB", [16, NCH, 128], BF16, kind="ExternalInput")
    yT = nc.dram_tensor("yT", [D, NPOS], F32, kind="ExternalOutput")

    with tile.TileContext(nc) as tc:
        with (
            tc.tile_pool(name="const", bufs=1) as cpool,
            tc.tile_pool(name="wa", bufs=3) as wapool,
            tc.tile_pool(name="wb", bufs=3) as wbpool,
            tc.tile_pool(name="xt", bufs=3) as xtpool,
            tc.tile_pool(name="pt", bufs=1) as ptpool,
            tc.tile_pool(name="qk", bufs=1) as qkpool,
            tc.tile_pool(name="pr", bufs=2) as prpool,
            tc.tile_pool(name="sm", bufs=1) as smpool,
            tc.tile_pool(name="fz", bufs=1) as fzpool,
            tc.tile_pool(name="yo", bufs=2) as yopool,
            tc.tile_pool(name="ps", bufs=1, space="PSUM") as pspool,
        ):
            def psum(i, shape=(128, BLK)):
                return pspool.tile(list(shape), F32, tag=f"a{i}", name=f"ps_a{i}")

            # ---- constants ----
            selw = cpool.tile([128, 127], BF16, tag="selw")
            nc.sync.dma_start(selw[:], selw_d[:])
            selA = cpool.tile([64, M * WIN, 16], BF16, tag="selA")
            nc.sync.dma_start(selA[:], selA_d[:])
            selB = cpool.tile([16, NCH, 128], BF16, tag="selB")
            nc.sync.dma_start(selB[:], selB_d[:])
            bmt = cpool.tile([128, M, NCH], F32, tag="bmt")
            nc.sync.dma_start(bmt[:], bm_d[:])
            embt = cpool.tile([128, M, NCH], F32, tag="embt")
            nc.sync.dma_start(embt[:], emb_d[:])
            biasA = cpool.tile([128, M, NCH], F32, tag="biasA")
            nc.vector.tensor_add(biasA[:], bmt[:], embt[:])
            bqkvo = {}
            for nm, dd in (("bq", bq_d), ("bk", bk_d), ("bv", bv_d), ("bo", bo_d)):
                t = cpool.tile([128, NCH], F32, tag=nm)
                nc.sync.dma_start(t[:], dd[:])
                bqkvo[nm] = t

            import contextlib
            rep_cm = (tc.For_i(0, repeat, 1,
                               hint_engines=(mybir.EngineType.PE,
                                             mybir.EngineType.Activation,
                                             mybir.EngineType.DVE,
                                             mybir.EngineType.SP,
                                             mybir.EngineType.Pool))
                      if repeat > 1 else contextlib.nullcontext())
            pending_D = []
            with rep_cm:
                for blk in range(NBLK):
                    nact = [n_active(m, blk) for m in range(M)]
                    act_m = [m for m in range(M) if nact[m] > 0]
                    act_pairs = [(m, w, r) for (m, w, r) in PAIRS
                                 if nact[m] > 0 and nact[r] > 0]
                    p0 = blk * BLK

                    # ---------- stage A ----------
                    pT = {}
                    for m in act_m:
                        na = nact[m]
                        nk = MOD[m][1] // 128
                        pt = ptpool.tile([128, NCH, BLK], BF16, tag=f"pt{m}")
                        pT[m] = pt
                        if na < BLK:
                            nc.gpsimd.memset(pt[:, :, na:BLK], 0.0)
                        accs = [psum(i) for i in range(NCH)]
                        for dk in range(nk):
                            deng = nc.sync if dk % 2 == 0 else nc.scalar
                            wrow = wapool.tile([128, D], BF16, tag="wa")
                            deng.dma_start(
                                wrow[:], WmT[m][dk * 128:(dk + 1) * 128, :])
                            xt_t = xtpool.tile([128, BLK], BF16, tag="xt")
                            deng.dma_start(
                                xt_t[:, :na],
                                xT[m][dk * 128:(dk + 1) * 128, p0:p0 + na])
                            for dc in range(NCH):
                                nc.tensor.matmul(
                                    accs[dc][:, :na],
                                    wrow[:, dc * 128:(dc + 1) * 128],
                                    xt_t[:, :na],
                                    start=(dk == 0), stop=(dk == nk - 1),
                                    skip_group_check=True)
                        for dc in range(NCH):
                            nc.scalar.activation(
                                pt[:, dc, :na], accs[dc][:, :na], AF.Identity,
                                bias=biasA[:, m, dc:dc + 1])

                    # ---------- pass 1: q, k, scores ----------
                    n_sc = {w: sum(1 for (m, w2, r) in act_pairs if w2 == w) * NCH
                            for w in range(WIN)}
                    c_sc = {w: 0 for w in range(WIN)}
                    sc_ps = [psum(5 + w, (64, BLK)) if n_sc[w] > 0 else None
                             for w in range(WIN)]
                    def emit_qk(c):
                        qk_t = {}
                        for ti, (tname, wd, bias) in enumerate(
                                (("q", WqT, bqkvo["bq"]),
                                 ("k", WkT, bqkvo["bk"]))):
                            wsl = wbpool.tile([128, NCH, 128], BF16, tag="wb",
                                              name="wsl")
                            nc.sync.dma_start(wsl[:], wd[c])
                            for mi, m in enumerate(act_m):
                                acc = psum((2 * c + ti + mi) % 3)
                                for dk in range(NCH):
                                    nc.tensor.matmul(
                                        acc[:], wsl[:, dk, :], pT[m][:, dk, :],
                                        start=(dk == 0), stop=(dk == NCH - 1),
                                        skip_group_check=True)
                                t = qkpool.tile([128, BLK], BF16,
                                                tag=f"{tname}{m}_{c % 2}",
                                                name=f"{tname}{m}")
                                nc.scalar.activation(
                                    t[:], acc[:], AF.Identity,
                                    bias=bias[:, c:c + 1])
                                qk_t[(tname, m)] = t
                        return qk_t

                    def emit_scores(c, qk_t):
                        for pi, (m, w, r) in enumerate(act_pairs):
                            prod = prpool.tile([128, BLK], BF16, bufs=1,
                                               tag=f"prod{pi}", name="prod")
                            nc.vector.tensor_mul(
                                prod[:], qk_t[("q", m)][:], qk_t[("k", r)][:])
                            off = 62 - (16 * m + 2 * c)
                            i = c_sc[w]
                            c_sc[w] += 1
                            nc.tensor.matmul(
                                sc_ps[w][:], selw[:, off:off + 64], prod[:],
                                start=(i == 0), stop=(i == n_sc[w] - 1),
                                skip_group_check=True)

                    prev = emit_qk(0)
                    for c in range(1, NCH):
                        cur = emit_qk(c)
                        emit_scores(c - 1, prev)
                        prev = cur
                    emit_scores(NCH - 1, prev)
                    if pending_D:
                        emit_stage_D(*pending_D.pop(0))

                    # ---------- softmax ----------
                    s_sb = []
                    for w in range(WIN):
                        t = smpool.tile([64, BLK], F32, tag=f"s{w}")
                        if sc_ps[w] is None:
                            nc.gpsimd.memset(t[:], 0.0)
                        else:
                            nc.vector.tensor_copy(t[:], sc_ps[w][:])
                        s_sb.append(t)
                    mx = smpool.tile([64, BLK], F32, tag="mx")
                    nc.vector.tensor_tensor(mx[:], s_sb[0][:], s_sb[1][:], op=ALU.max)
                    nc.vector.tensor_tensor(mx[:], mx[:], s_sb[2][:], op=ALU.max)
                    attn = []
                    for w in range(WIN):
                        nc.vector.tensor_tensor(s_sb[w][:], s_sb[w][:], mx[:],
                                                op=ALU.subtract)
                        a = smpool.tile([64, BLK], F32R, tag=f"at{w}")
                        nc.scalar.activation(a[:], s_sb[w][:], AF.Exp, scale=scale)
                        attn.append(a)
                    den = smpool.tile([64, BLK], F32, tag="mx")
                    nc.vector.tensor_add(den[:], attn[0][:], attn[1][:])
                    nc.vector.tensor_add(den[:], den[:], attn[2][:])
                    rec = smpool.tile([64, BLK], F32R, tag="rec")
                    with nc.allow_low_precision(reason="fp32r attn weights"):
                        nc.vector.reciprocal(rec[:], den[:])
                    attn_b = []
                    for w in range(WIN):
                        ab_t = smpool.tile([64, BLK], BF16, tag=f"ab{w}")
                        with nc.allow_low_precision(reason="fp16 attn"):
                            nc.vector.tensor_mul(ab_t[:], attn[w][:], rec[:])
                        attn_b.append(ab_t)
                    attn = attn_b

                    # ---------- pass 2: v, A16, Abc, fused ----------
                    act_r = [r for r in range(M) if nact[r] > 0]
                    fz = fzpool.tile([128, NCH, BLK], BF16, tag="fz")

                    def emit_v(c):
                        wsl = wbpool.tile([128, NCH, 128], BF16, tag="wb",
                                          name="wslv")
                        nc.sync.dma_start(wsl[:], WvT[c])
                        v_t = {}
                        for mi, m in enumerate(act_m):
                            acc = psum([0, 5, 6][mi % 3])
                            for dk in range(NCH):
                                nc.tensor.matmul(
                                    acc[:], wsl[:, dk, :], pT[m][:, dk, :],
                                    start=(dk == 0), stop=(dk == NCH - 1),
                                    skip_group_check=True)
                            t = qkpool.tile([128, BLK], BF16,
                                            tag=f"q{m}_{c % 2}", name="vt")
                            nc.scalar.activation(
                                t[:], acc[:], AF.Identity,
                                bias=bqkvo["bv"][:, c:c + 1])
                            v_t[m] = t
                        return v_t

                    def emit_fused(c, v_t, a16sb):
                        ab_ps = {}
                        for r in act_r:
                            ab = psum(1 + r)
                            nc.tensor.matmul(
                                ab[:], selB[:, c, :], a16sb[:, r, :],
                                start=True, stop=True,
                                skip_group_check=True)
                            ab_ps[r] = ab
                        r0 = act_r[0]
                        accv = prpool.tile([128, BLK], F32, tag="f0",
                                           name="accv")
                        nc.vector.tensor_mul(accv[:], ab_ps[r0][:], v_t[r0][:])
                        if len(act_r) == 1:
                            nc.vector.tensor_copy(fz[:, c, :], accv[:])
                        for j, r in enumerate(act_r[1:]):
                            tmp = prpool.tile([128, BLK], F32, tag="f1",
                                              name="tmp")
                            nc.vector.tensor_mul(tmp[:], ab_ps[r][:], v_t[r][:])
                            last = (j == len(act_r) - 2)
                            nc.vector.tensor_add(
                                fz[:, c, :] if last else accv[:],
                                accv[:], tmp[:])

                    vbuf = {0: emit_v(0)}
                    if NCH > 1:
                        vbuf[1] = emit_v(1)

                    # A16 = per-source summed attn (waits on softmax; emitted
                    # after two v chunks so the PE stream has work meanwhile)
                    a16sb = smpool.tile([16, M, BLK], BF16, tag="a16sb")
                    for r in act_r:
                        a16 = psum(1 + r, (16, BLK))
                        srcs = SRC[r]
                        for i, (m, w) in enumerate(srcs):
                            nc.tensor.matmul(
                                a16[:], selA[:, PAIR_IDX[(m, w)], :],
                                attn[w][:],
                                start=(i == 0), stop=(i == len(srcs) - 1),
                                skip_group_check=True)
                        nc.scalar.activation(a16sb[:, r, :], a16[:], AF.Identity)

                    for c in range(NCH):
                        emit_fused(c, vbuf.pop(c), a16sb)
                        if c + 2 < NCH:
                            vbuf[c + 2] = emit_v(c + 2)

                    # ---------- stage D (deferred) ----------
                    def emit_stage_D(fz, p0):
                        for dc in range(NCH):
                            wsl = wbpool.tile([128, NCH, 128], BF16,
                                              tag="wb", name="wsld")
                            nc.sync.dma_start(wsl[:], WoT[dc])
                            acc = psum(dc % 3)
                            for dk in range(NCH):
                                nc.tensor.matmul(
                                    acc[:], wsl[:, dk, :], fz[:, dk, :],
                                    start=(dk == 0), stop=(dk == NCH - 1),
                                    skip_group_check=True)
                            yo = yopool.tile([128, BLK], F32, tag="yo")
                            nc.scalar.activation(yo[:], acc[:], AF.Identity,
                                                 bias=bqkvo["bo"][:, dc:dc + 1])
                            nc.sync.dma_start(
                                yT[dc * 128:(dc + 1) * 128, p0:p0 + BLK], yo[:])

                    pending_D.append((fz, p0))

                    if blk == NBLK - 1:
                        while pending_D:
                            emit_stage_D(*pending_D.pop(0))


    nc.compile()
    _BUILD_CACHE[key] = nc
    return nc


import ml_dtypes

BF16_NP = np.float16


def make_selw():
    sw = np.zeros((128, 127), np.float32)
    for p in range(128):
        sw[p, 62 + p // 64] = 1.0
    return sw


def make_selA():
    sa = np.zeros((64, M * WIN, 16), np.float32)
    for m in range(M):
        for w in range(WIN):
            for h in range(16):
                sa[16 * m + h, m * WIN + w, h] = 1.0
    return sa


def make_selB():
    sb = np.zeros((16, NCH, 128), np.float32)
    for c in range(NCH):
        for j in range(128):
            sb[2 * c + j // 64, c, j] = 0.25
    return sb


def _vec_tile(v):
    return np.ascontiguousarray(np.asarray(v, np.float32).reshape(NCH, 128).T)


def prepare_in_maps(inputs):
    names = [mm[0] for mm in MOD]
    shared = {}
    for i, nm in enumerate(names):
        shared[f"WmT{i}"] = np.ascontiguousarray(
            np.asarray(inputs[f"W_{nm}"], np.float32).T).astype(BF16_NP)
    for k, srcn in (("WqT", "Wq"), ("WkT", "Wk"), ("WvT", "Wv"), ("WoT", "Wo")):
        wt = np.asarray(inputs[srcn], np.float32).T      # [din, dout]
        wt = wt.reshape(NCH, 128, NCH, 128)              # [dk, p, c, j]
        shared[k] = np.ascontiguousarray(
            wt.transpose(2, 1, 0, 3)).astype(BF16_NP)    # [c, p, dk, j]
    shared["bm"] = np.ascontiguousarray(
        np.stack([_vec_tile(inputs[f"b_{nm}"]) for nm in names], axis=1))
    shared["emb"] = np.ascontiguousarray(
        np.stack([_vec_tile(np.asarray(inputs["mod_emb"])[i]) for i in range(M)],
                 axis=1))
    for k in ("bq", "bk", "bv", "bo"):
        shared[k] = _vec_tile(inputs[k])
    shared["selw"] = make_selw().astype(BF16_NP)
    shared["selA"] = make_selA().astype(BF16_NP)
    shared["selB"] = make_selB().astype(BF16_NP)

    in_maps = []
    for core in range(8):
        b, par = core // 2, core % 2
        im = dict(shared)
        for i, nm in enumerate(names):
            x = np.asarray(inputs[nm], np.float32)[b, par::2][:NLOC[i]]
            im[f"xT{i}"] = np.ascontiguousarray(x.T).astype(BF16_NP)
        in_maps.append(im)
    return in_maps


def kernel(**inputs):
    inputs = {k: np.asarray(v) for k, v in inputs.items()}
    scale = float(1.0 / (np.sqrt(HD) * abs(float(inputs["temperature"]))))
    nc = build(scale, repeat=1)
    in_maps = prepare_in_maps(inputs)
    res = run_bass_kernel_spmd(nc, in_maps, list(range(8)))
    out = np.zeros((B, S, D), np.float32)
    for core in range(8):
        b, par = core // 2, core % 2
        out[b, par::2, :] = res.results[core]["yT"].T
    return out



# revision 8
# speedup vs baseline: 1.6756x; 1.0814x over previous
"""Trainium2 Bass kernel for nn_CantorModalityFusion — v2 (folded QKV).

Sharding: 8 cores = (batch b in 0..3) x (position parity in 0..1); each core
handles 1024 positions of one batch. Computation is per-(b, s) independent
-> no collectives.

Structure (per core, per repeat iteration):
  F_qk: per modality m (video, image, text, audio):
        text/image/audio: q,k = (Wq@Wm)ᵀ-folded fp16 matmuls straight from x
        video (nk=16, folding loses): pt = Wmᵀ@x, then q,k = Wq/Wk @ pt
        -> q/k stores [128, NCH, NLOC_pad[m]] fp16 in SBUF
  S:    per 256-range: prod = q*k (DVE), scores = selwᵀ@prod (PE, PSUM acc
        over chunks), softmax (DVE/ACT), A16[r] = selAᵀ@attn (PE) -> a16sb
  F_v:  per modality m: v tiles from folded (or video two-stage) matmuls,
        Abc = selBᵀ@a16 (PE), fz[:, c, rng] (+)= Abc * v (DVE)
  D:    y = Woᵀ @ fz (+bo) per 512-block, DMA out.

All matmul operands fp16 (1 cycle/row on PE, half DMA); PSUM/softmax f32.
Folded per-modality cost: 24*nk rows vs nk*8+192 unfolded -> fold iff nk<12
(text 6, image 8, audio 4 yes; video 16 no).
"""

import sys

import numpy as np

sys.path.insert(0, "/opt/trn_rl_repo")

import concourse.bacc as bacc
import concourse.mybir as mybir
from concourse import tile
from concourse.bass_utils import run_bass_kernel_spmd

F32 = mybir.dt.float32
F32R = mybir.dt.float32r
FP16 = mybir.dt.float16
AF = mybir.ActivationFunctionType
ALU = mybir.AluOpType

B, S, D, H, HD = 4, 2048, 1024, 16, 64
M, WIN = 4, 3
MOD = [("text", 768, 2048), ("image", 1024, 1024), ("audio", 512, 1500), ("video", 2048, 512)]
ROUTES = [[0, 1, 2], [0, 1, 2], [2, 3, 0], [3, 2, 0]]
PAIRS = [(m, w, ROUTES[m][w]) for m in range(M) for w in range(WIN)]
SRC = {r: [(m, w) for (m, w, rr) in PAIRS if rr == r] for r in range(M)}
PAIR_IDX = {(m, w): m * WIN + w for m in range(M) for w in range(WIN)}

NPOS = S // 2
NCH = D // 128                            # 8 feature chunks
NLOC = [sl // 2 for (_, _, sl) in MOD]    # 1024, 512, 750, 256
NLOC_PAD = [1024, 512, 768, 256]
NK = [dim // 128 for (_, dim, _) in MOD]  # 6, 8, 4, 16
RNG = 256
NRNG = NPOS // RNG                        # 4
PROC = [3, 1, 0, 2]                       # video, image, text, audio
FOLD = [0, 1, 2]                          # folded modalities

_BUILD_CACHE = {}


def mod_active(m, rng):
    return NLOC[m] > rng * RNG


def rng_pairs(rng):
    return [(m, w, r) for (m, w, r) in PAIRS
            if mod_active(m, rng) and mod_active(r, rng)]


def posblocks(m):
    """512-wide (last may be 256) position blocks covering NLOC_PAD[m]."""
    out, p = [], 0
    while p < NLOC_PAD[m]:
        w = min(512, NLOC_PAD[m] - p)
        out.append((p, w))
        p += w
    return out


def build(scale, repeat=1):
    key = (float(scale), repeat)
    if key in _BUILD_CACHE:
        return _BUILD_CACHE[key]
    nc = bacc.Bacc("TRN2", target_bir_lowering=False, debug=False)

    x_d = [nc.dram_tensor(f"x{m}", [NK[m] * 128, NLOC[m]], FP16,
                          kind="ExternalInput") for m in range(M)]
    # folded q,k weights per modality: [2, nk, 128, NCH, 128]
    wqk_d = {m: nc.dram_tensor(f"wqk{m}", [2, NK[m], 128, NCH, 128], FP16,
                               kind="ExternalInput") for m in FOLD}
    # folded v weights per modality: [nk, 128, NCH, 128]
    wv_d = {m: nc.dram_tensor(f"wv{m}", [NK[m], 128, NCH, 128], FP16,
                              kind="ExternalInput") for m in FOLD}
    # video two-stage weights
    wmv_d = nc.dram_tensor("wmv", [16, 128, NCH, 128], FP16, kind="ExternalInput")
    wqv_d = nc.dram_tensor("wqv", [NCH, 128, NCH, 128], FP16, kind="ExternalInput")
    wkv_d = nc.dram_tensor("wkv", [NCH, 128, NCH, 128], FP16, kind="ExternalInput")
    wvv_d = nc.dram_tensor("wvv", [NCH, 128, NCH, 128], FP16, kind="ExternalInput")
    wo_d = nc.dram_tensor("wo", [NCH, 128, NCH, 128], FP16, kind="ExternalInput")
    # biases: bias3[proj, m] folded (plain bq/bk/bv for video); biasAv video
    bias3_d = nc.dram_tensor("bias3", [128, 3, M, NCH], F32, kind="ExternalInput")
    biasAv_d = nc.dram_tensor("biasAv", [128, NCH], F32, kind="ExternalInput")
    bo_d = nc.dram_tensor("bo", [128, NCH], F32, kind="ExternalInput")
    selw_d = nc.dram_tensor("selw", [128, 127], FP16, kind="ExternalInput")
    selA_d = nc.dram_tensor("selA", [64, M * WIN, 16], FP16, kind="ExternalInput")
    selB_d = nc.dram_tensor("selB", [16, NCH, 128], FP16, kind="ExternalInput")
    yT = nc.dram_tensor("yT", [D, NPOS], F32, kind="ExternalOutput")

    with tile.TileContext(nc) as tc:
        with (
            tc.tile_pool(name="const", bufs=1) as cpool,
            tc.tile_pool(name="w3", bufs=11) as w3pool,     # 256KB slices
            tc.tile_pool(name="xs", bufs=1) as xpool,
            tc.tile_pool(name="qk", bufs=1) as qkpool,
            tc.tile_pool(name="ptv", bufs=1) as ptvpool,
            tc.tile_pool(name="vt", bufs=3) as vtpool,
            tc.tile_pool(name="pr", bufs=4) as prpool,
            tc.tile_pool(name="sm", bufs=1) as smpool,
            tc.tile_pool(name="fz", bufs=1) as fzpool,
            tc.tile_pool(name="yo", bufs=2) as yopool,
            tc.tile_pool(name="ps", bufs=1, space="PSUM") as pspool,
        ):
            def psum(i, shape):
                return pspool.tile(list(shape), F32, tag=f"a{i}", name=f"ps_a{i}")

            # ---- constants (outside repeat loop) ----
            selw = cpool.tile([128, 127], FP16, tag="selw")
            nc.sync.dma_start(selw[:], selw_d[:])
            selA = cpool.tile([64, M * WIN, 16], FP16, tag="selA")
            nc.sync.dma_start(selA[:], selA_d[:])
            selB = cpool.tile([16, NCH, 128], FP16, tag="selB")
            nc.sync.dma_start(selB[:], selB_d[:])
            bias3 = cpool.tile([128, 3, M, NCH], F32, tag="bias3")
            nc.sync.dma_start(bias3[:], bias3_d[:])
            biasAv = cpool.tile([128, NCH], F32, tag="biasAv")
            nc.sync.dma_start(biasAv[:], biasAv_d[:])
            bo = cpool.tile([128, NCH], F32, tag="bo")
            nc.sync.dma_start(bo[:], bo_d[:])

            import contextlib
            rep_cm = (tc.For_i(0, repeat, 1,
                               hint_engines=(mybir.EngineType.PE,
                                             mybir.EngineType.Activation,
                                             mybir.EngineType.DVE,
                                             mybir.EngineType.SP,
                                             mybir.EngineType.Pool))
                      if repeat > 1 else contextlib.nullcontext())

            dmac = [0]

            def dma(dst, src):
                eng = nc.sync if dmac[0] % 2 == 0 else nc.scalar
                dmac[0] += 1
                eng.dma_start(dst, src)

            def w3slice(src):
                t = w3pool.tile([128, NCH, 128], FP16, tag="w3", name="w3s")
                dma(t[:], src)
                return t

            with rep_cm:
                # ================= F_qk =================
                qs, ks = {}, {}
                for m in range(M):
                    qs[m] = qkpool.tile([128, NCH, NLOC_PAD[m]], FP16, tag=f"q{m}", name=f"qs{m}")
                    ks[m] = qkpool.tile([128, NCH, NLOC_PAD[m]], FP16, tag=f"k{m}", name=f"ks{m}")

                # x tiles (resident; reused by F_v); DMA issued per modality
                # just before its compute so queue order pipelines cleanly
                xs = {}

                def load_x(m):
                    xt = xpool.tile([128, NK[m], NLOC_PAD[m]], FP16, tag=f"x{m}", name=f"xs{m}")
                    xs[m] = xt
                    if NLOC[m] < NLOC_PAD[m]:
                        nc.gpsimd.memset(xt[:, :, NLOC[m]:], 0.0)
                    for dk in range(NK[m]):
                        dma(xt[:, dk, :NLOC[m]],
                            x_d[m][dk * 128:(dk + 1) * 128, :])

                # --- video two-stage: pt then q,k ---
                load_x(3)
                ptv = ptvpool.tile([128, NCH, 256], FP16, tag="ptv")
                vacc = [psum(i, (128, 256)) for i in range(NCH)]
                for dk in range(16):
                    wsl = w3slice(wmv_d[dk])
                    for c in range(NCH):
                        nc.tensor.matmul(vacc[c][:], wsl[:, c, :],
                                         xs[3][:, dk, :],
                                         start=(dk == 0), stop=(dk == 15),
                                         skip_group_check=True)
                for c in range(NCH):
                    nc.scalar.activation(ptv[:, c, :], vacc[c][:], AF.Identity,
                                         bias=biasAv[:, c:c + 1])
                for pi, (wd, store) in enumerate(((wqv_d, qs[3]), (wkv_d, ks[3]))):
                    for c in range(NCH):
                        wsl = w3slice(wd[c])
                        acc = psum((2 * c + pi) % 4, (128, 256))
                        for dk in range(NCH):
                            nc.tensor.matmul(acc[:], wsl[:, dk, :], ptv[:, dk, :],
                                             start=(dk == 0), stop=(dk == NCH - 1),
                                             skip_group_check=True)
                        nc.scalar.activation(store[:, c, :], acc[:], AF.Identity,
                                             bias=bias3[:, pi, 3, c:c + 1])

                # --- folded q,k for image, text, audio ---
                for m in [1, 0, 2]:
                    nk = NK[m]
                    load_x(m)
                    for pi, store in ((0, qs[m]), (1, ks[m])):
                        wsl = [w3slice(wqk_d[m][pi, dk]) for dk in range(nk)]
                        for c in range(NCH):
                            for bi, (p0, pw) in enumerate(posblocks(m)):
                                acc = psum((2 * c + bi + pi) % 4, (128, pw))
                                for dk in range(nk):
                                    nc.tensor.matmul(
                                        acc[:], wsl[dk][:, c, :],
                                        xs[m][:, dk, p0:p0 + pw],
                                        start=(dk == 0), stop=(dk == nk - 1),
                                        skip_group_check=True)
                                nc.scalar.activation(
                                    store[:, c, p0:p0 + pw], acc[:], AF.Identity,
                                    bias=bias3[:, pi, m, c:c + 1])
                    if NLOC[m] < NLOC_PAD[m]:
                        nc.gpsimd.memset(qs[m][:, :, NLOC[m]:], 0.0)
                        nc.gpsimd.memset(ks[m][:, :, NLOC[m]:], 0.0)

                # ================= S =================
                a16sb = cpool.tile([16, M, NPOS], FP16, tag="a16sb", name="a16sb")

                def emit_scores(rng):
                    pairs = rng_pairs(rng)
                    o = rng * RNG
                    n_sc = {w: sum(1 for (_, w2, _) in pairs if w2 == w) * NCH
                            for w in range(WIN)}
                    c_sc = {w: 0 for w in range(WIN)}
                    # alternate PSUM banks between adjacent ranges so range
                    # r+1's scores don't WAR-stall on r's softmax reads;
                    # r3 must avoid a0..a3 (reused by F_v(video) before
                    # sm_a16(r3) in PE order -> would deadlock DVE/PE)
                    base = {0: 5, 1: 1, 2: 5, 3: 4}[rng]
                    sc_ps = {w: psum(base + w, (64, RNG)) for w in range(WIN)
                             if n_sc[w] > 0}
                    for c in range(NCH):
                        for (m, w, r) in pairs:
                            prod = prpool.tile([128, RNG], FP16, tag="prod",
                                               name="prod")
                            nc.vector.tensor_mul(prod[:],
                                                 qs[m][:, c, o:o + RNG],
                                                 ks[r][:, c, o:o + RNG])
                            off = 62 - (16 * m + 2 * c)
                            i = c_sc[w]
                            c_sc[w] += 1
                            nc.tensor.matmul(
                                sc_ps[w][:], selw[:, off:off + 64], prod[:],
                                start=(i == 0), stop=(i == n_sc[w] - 1),
                                skip_group_check=True)
                    return sc_ps

                def emit_sm_a16(rng, sc_ps):
                    o = rng * RNG
                    s_sb = []
                    for w in range(WIN):
                        t = smpool.tile([64, RNG], F32, tag=f"s{w}", name="s_sb")
                        if w in sc_ps:
                            nc.vector.tensor_copy(t[:], sc_ps[w][:])
                        else:
                            nc.gpsimd.memset(t[:], 0.0)
                        s_sb.append(t)
                    mx = smpool.tile([64, RNG], F32, tag="mx")
                    nc.vector.tensor_tensor(mx[:], s_sb[0][:], s_sb[1][:],
                                            op=ALU.max)
                    nc.vector.tensor_tensor(mx[:], mx[:], s_sb[2][:], op=ALU.max)
                    es = []
                    for w in range(WIN):
                        nc.vector.tensor_tensor(s_sb[w][:], s_sb[w][:], mx[:],
                                                op=ALU.subtract)
                        e = smpool.tile([64, RNG], F32R, tag=f"e{w}", name="e_w")
                        nc.scalar.activation(e[:], s_sb[w][:], AF.Exp, scale=scale)
                        es.append(e)
                    den = smpool.tile([64, RNG], F32, tag="mx")
                    nc.vector.tensor_add(den[:], es[0][:], es[1][:])
                    nc.vector.tensor_add(den[:], den[:], es[2][:])
                    rec = smpool.tile([64, RNG], F32R, tag="rec")
                    with nc.allow_low_precision(reason="fp16 attn weights"):
                        nc.vector.reciprocal(rec[:], den[:])
                    attn = []
                    for w in range(WIN):
                        a = smpool.tile([64, RNG], FP16, tag=f"at{w}", name="attn")
                        nc.vector.tensor_mul(a[:], es[w][:], rec[:])
                        attn.append(a)
                    for r in range(M):
                        if not mod_active(r, rng):
                            continue
                        a16 = psum(4, (16, RNG))
                        srcs = SRC[r]
                        for i, (m, w) in enumerate(srcs):
                            nc.tensor.matmul(
                                a16[:], selA[:, PAIR_IDX[(m, w)], :], attn[w][:],
                                start=(i == 0), stop=(i == len(srcs) - 1),
                                skip_group_check=True)
                        nc.scalar.activation(a16sb[:, r, o:o + RNG], a16[:],
                                             AF.Identity)

                sc0 = emit_scores(0)
                sc1 = emit_scores(1)
                emit_sm_a16(0, sc0)
                sc2 = emit_scores(2)
                emit_sm_a16(1, sc1)
                sc3 = emit_scores(3)
                emit_sm_a16(2, sc2)

                # ================= F_v (+ fused accumulate) =================
                fz = fzpool.tile([128, NCH, NPOS], FP16, tag="fz")

                def fused_acc(m, c, p0, pw, vt):
                    """fz[:, c, p0:p0+pw] (+)= Abc(m) * vt, split per 256-rng
                    so the first active modality writes, later ones add."""
                    ab = psum(2 + (c % 2), (128, pw))
                    nc.tensor.matmul(ab[:], selB[:, c, :],
                                     a16sb[:, m, p0:p0 + pw],
                                     start=True, stop=True,
                                     skip_group_check=True)
                    for rng in range(p0 // RNG, (p0 + pw) // RNG):
                        ro = rng * RNG - p0
                        first = PROC.index(m) == min(
                            PROC.index(r) for r in range(M)
                            if mod_active(r, rng))
                        dst = fz[:, c, rng * RNG:(rng + 1) * RNG]
                        if first:
                            nc.vector.tensor_mul(
                                dst, ab[:, ro:ro + RNG], vt[:, ro:ro + RNG])
                        else:
                            tmp = prpool.tile([128, RNG], F32, tag="ftmp",
                                              name="ftmp", bufs=2)
                            nc.vector.tensor_mul(
                                tmp[:], ab[:, ro:ro + RNG], vt[:, ro:ro + RNG])
                            nc.vector.tensor_add(dst, dst, tmp[:])

                # video v from ptv
                for c in range(NCH):
                    wsl = w3slice(wvv_d[c])
                    acc = psum(c % 2, (128, 256))
                    for dk in range(NCH):
                        nc.tensor.matmul(acc[:], wsl[:, dk, :], ptv[:, dk, :],
                                         start=(dk == 0), stop=(dk == NCH - 1),
                                         skip_group_check=True)
                    vt = vtpool.tile([128, 512], FP16, tag="vt", name="vt")
                    nc.scalar.activation(vt[:, :256], acc[:], AF.Identity,
                                         bias=bias3[:, 2, 3, c:c + 1])
                    fused_acc(3, c, 0, 256, vt)

                # r3 softmax hides behind F_v(video) PE work; its A16 only
                # feeds F_v(text) (r3 is text-only)
                emit_sm_a16(3, sc3)

                # folded v for image, text, audio
                for m in [1, 0, 2]:
                    nk = NK[m]
                    wsl = [w3slice(wv_d[m][dk]) for dk in range(nk)]
                    for bi, (p0, pw) in enumerate(posblocks(m)):
                        for c in range(NCH):
                            acc = psum(c % 2, (128, pw))
                            for dk in range(nk):
                                nc.tensor.matmul(
                                    acc[:], wsl[dk][:, c, :],
                                    xs[m][:, dk, p0:p0 + pw],
                                    start=(dk == 0), stop=(dk == nk - 1),
                                    skip_group_check=True)
                            vt = vtpool.tile([128, 512], FP16, tag="vt",
                                             name="vt")
                            nc.scalar.activation(
                                vt[:, :pw], acc[:], AF.Identity,
                                bias=bias3[:, 2, m, c:c + 1])
                            if NLOC[m] < p0 + pw:
                                nc.gpsimd.memset(vt[:, NLOC[m] - p0:pw], 0.0)
                            fused_acc(m, c, p0, pw, vt)

                # ================= D =================
                for blk in range(2):
                    p0 = blk * 512
                    for c in range(NCH):
                        wsl = w3slice(wo_d[c])
                        acc = psum(4 + (c % 2), (128, 512))
                        for dk in range(NCH):
                            nc.tensor.matmul(
                                acc[:], wsl[:, dk, :], fz[:, dk, p0:p0 + 512],
                                start=(dk == 0), stop=(dk == NCH - 1),
                                skip_group_check=True)
                        yo = yopool.tile([128, 512], F32, tag="yo")
                        nc.scalar.activation(yo[:], acc[:], AF.Identity,
                                             bias=bo[:, c:c + 1])
                        dma(yT[c * 128:(c + 1) * 128, p0:p0 + 512], yo[:])

    nc.compile()
    _BUILD_CACHE[key] = nc
    return nc


def make_selw():
    sw = np.zeros((128, 127), np.float32)
    for p in range(128):
        sw[p, 62 + p // 64] = 1.0
    return sw


def make_selA():
    sa = np.zeros((64, M * WIN, 16), np.float32)
    for m in range(M):
        for w in range(WIN):
            for h in range(16):
                sa[16 * m + h, m * WIN + w, h] = 1.0
    return sa


def make_selB():
    sb = np.zeros((16, NCH, 128), np.float32)
    for c in range(NCH):
        for j in range(128):
            sb[2 * c + j // 64, c, j] = 0.25
    return sb


def _vec_tile(v):
    return np.ascontiguousarray(np.asarray(v, np.float32).reshape(NCH, 128).T)


def _wslices(w):
    """[dout, din] weight -> [nk, 128, NCH, 128] fp16 (wᵀ reshaped)."""
    wt = np.ascontiguousarray(np.asarray(w, np.float32).T)   # [din, dout]
    nk = wt.shape[0] // 128
    return np.ascontiguousarray(
        wt.reshape(nk, 128, NCH, 128)).astype(np.float16)


def _cslices(w):
    """[dout, din] -> [c, p, dk, j] fp16 (per-out-chunk stationary slices)."""
    wt = np.asarray(w, np.float32).T
    nk = wt.shape[0] // 128
    wt = wt.reshape(nk, 128, NCH, 128)
    return np.ascontiguousarray(wt.transpose(2, 1, 0, 3)).astype(np.float16)


def prepare_in_maps(inputs):
    names = [mm[0] for mm in MOD]
    Wq = np.asarray(inputs["Wq"], np.float32)
    Wk = np.asarray(inputs["Wk"], np.float32)
    Wv = np.asarray(inputs["Wv"], np.float32)
    shared = {}
    for m in FOLD:
        Wm = np.asarray(inputs[f"W_{names[m]}"], np.float32)
        shared[f"wqk{m}"] = np.stack(
            [_wslices(Wq @ Wm), _wslices(Wk @ Wm)], axis=0)
        shared[f"wv{m}"] = _wslices(Wv @ Wm)
    shared["wmv"] = _wslices(inputs["W_video"])
    shared["wqv"] = _cslices(Wq)
    shared["wkv"] = _cslices(Wk)
    shared["wvv"] = _cslices(Wv)
    shared["wo"] = _cslices(inputs["Wo"])

    emb = np.asarray(inputs["mod_emb"], np.float32)
    bias3 = np.zeros((128, 3, M, NCH), np.float32)
    for pi, (Wp, bp) in enumerate(((Wq, "bq"), (Wk, "bk"), (Wv, "bv"))):
        bpv = np.asarray(inputs[bp], np.float32)
        for m in range(M):
            bm = np.asarray(inputs[f"b_{names[m]}"], np.float32)
            if m in FOLD:
                vec = Wp @ (bm + emb[m]) + bpv
            else:
                vec = bpv
            bias3[:, pi, m, :] = _vec_tile(vec)
    shared["bias3"] = bias3
    shared["biasAv"] = _vec_tile(
        np.asarray(inputs["b_video"], np.float32) + emb[3])
    shared["bo"] = _vec_tile(inputs["bo"])
    shared["selw"] = make_selw().astype(np.float16)
    shared["selA"] = make_selA().astype(np.float16)
    shared["selB"] = make_selB().astype(np.float16)

    in_maps = []
    for core in range(8):
        b, par = core // 2, core % 2
        im = dict(shared)
        for i, nm in enumerate(names):
            x = np.asarray(inputs[nm], np.float32)[b, par::2][:NLOC[i]]
            im[f"x{i}"] = np.ascontiguousarray(x.T).astype(np.float16)
        in_maps.append(im)
    return in_maps


def kernel(**inputs):
    inputs = {k: np.asarray(v) for k, v in inputs.items()}
    scale = float(1.0 / (np.sqrt(HD) * abs(float(inputs["temperature"]))))
    nc = build(scale, repeat=1)
    in_maps = prepare_in_maps(inputs)
    res = run_bass_kernel_spmd(nc, in_maps, list(range(8)))
    out = np.zeros((B, S, D), np.float32)
    for core in range(8):
        b, par = core // 2, core % 2
        out[b, par::2, :] = res.results[core]["yT"].T
    return out
